# revision 1
# baseline (speedup 1.0000x reference)
"""Trainium2 Bass kernel for nn_CINLayer: out[b,d,o] = sum_{n,m} x[b,d,n]*y[b,d,m]*W[o,n*M+m].

Strategy (8-core data parallel over batch):
  Per sample s, out[o,s] = sum_k Wl[k,o] * Z[k,s] with Z[k,s] = x[s,n(k)]*y[s,m(k)].
  The contraction k (1600 products) is split into 13 chunks of 128 rows whose
  row->(n,m) mapping is chosen so each chunk's X-factor tile is a single
  DVE stream_shuffle of a host-staged interleaved layout Xil (per-quadrant
  lane-broadcast), and the Y-factor tiles are host-staged replicated layouts.
  Z chunks are built as one shuffle + one fp16 tensor_mul, then fed as the
  moving operand of fp16 matmuls accumulating out^T[o, s] in PSUM
  (o split 128+72, s tiles of 512).

  Chunk row mapping (r = 32j + r', j=quadrant):
    Part A (c<10):  (n, m) = (4c + j, r')          for r' < 32
    Part B (cb=c-10<3): r' = 8a + m''; (n, m) = (16cb + 4a + j, 32 + m'')
  Host layouts:
    Xil[32j + i]  = xT[4i + j]   (i<10, else 0)
    YrepA[p]      = yT[p % 32]
    YrepB[p]      = yT[32 + p % 8]
  Shuffle masks: A: mask[r'] = c ; B: mask[r'] = 4*cb + r'//8.
  W rows with n >= 40 (part B overhang) are zeroed on host.
"""

import numpy as np

BS, DIM, N, M, O = 2048, 32, 40, 40, 200
NCORES = 8
S_PER_CORE = BS * DIM // NCORES  # 8192
S_TILE = 512
N_STILES_FULL = S_PER_CORE // S_TILE  # 16
NCHUNKS = 13  # 10 part-A + 3 part-B
F16 = np.float16

# chunks whose Z-multiply runs on GPSIMD instead of DVE. GPSIMD's tensor_mul
# is ~9x slower per op than DVE's, but running a few there in parallel with
# the DVE shuffle/mul stream measured fastest (190us vs 214us all-DVE).
GPSIMD_MULS = frozenset({2, 4, 6, 9, 11})


def _chunk_row_to_nm(c: int, r: int):
    """Global chunk c (0..12), row r (0..127) -> (n, m) or None (zero pad)."""
    j, rp = divmod(r, 32)
    if c < 10:
        return 4 * c + j, rp
    cb = c - 10
    a, mpp = divmod(rp, 8)
    n = 16 * cb + 4 * a + j
    if n >= N:
        return None
    return n, 32 + mpp


def _shuffle_mask(c: int):
    if c < 10:
        return [c] * 32
    cb = c - 10
    return [4 * cb + (rp // 8) for rp in range(32)]


def _stage_w(W: np.ndarray) -> np.ndarray:
    """W [O, N*M] f32 -> wl [128, NCHUNKS, O] f16 (lhsT layout per chunk)."""
    Wr = W.reshape(O, N, M)
    wl = np.zeros((128, NCHUNKS, O), dtype=F16)
    for c in range(NCHUNKS):
        for r in range(128):
            nm = _chunk_row_to_nm(c, r)
            if nm is not None:
                wl[r, c, :] = Wr[:, nm[0], nm[1]].astype(F16)
    return wl


def _stage_core_inputs(x_flat: np.ndarray, y_flat: np.ndarray):
    """x_flat, y_flat [S_PER_CORE, 40] f32 -> xil, yrepa, yrepb [128, S] f16."""
    xT = np.ascontiguousarray(x_flat.T).astype(F16)  # [40, S]
    yT = np.ascontiguousarray(y_flat.T).astype(F16)  # [40, S]
    s = xT.shape[1]
    xil = np.zeros((128, s), dtype=F16)
    for p in range(128):
        j, i = divmod(p, 32)[0], p % 32
        if i < 10:
            xil[p] = xT[4 * i + j]
    yrepa = yT[np.arange(128) % 32]
    yrepb = yT[32 + (np.arange(128) % 8)]
    return xil, np.ascontiguousarray(yrepa), np.ascontiguousarray(yrepb)


def build_nc(n_stiles: int = N_STILES_FULL, debug: bool = False):
    """Build the per-core Bass/Tile module. Returns (nc, names dict)."""
    import concourse.bass as bass
    import concourse.tile as tile
    from concourse import bacc, mybir
    from concourse.tile_rust import add_dep_helper

    f16 = mybir.dt.float16
    f32 = mybir.dt.float32
    s_len = n_stiles * S_TILE

    nc = bacc.Bacc("TRN2", target_bir_lowering=False, debug=debug)

    xil_d = nc.dram_tensor("xil", [128, s_len], f16, kind="ExternalInput")
    ya_d = nc.dram_tensor("yrepa", [128, s_len], f16, kind="ExternalInput")
    yb_d = nc.dram_tensor("yrepb", [128, s_len], f16, kind="ExternalInput")
    wl_d = nc.dram_tensor("wl", [128, NCHUNKS, O], f16, kind="ExternalInput")
    out_d = nc.dram_tensor("outt", [O, s_len], f16, kind="ExternalOutput")

    with tile.TileContext(nc) as tc:
        with (
            tc.tile_pool(name="wpool", bufs=1) as wpool,
            tc.tile_pool(name="inp", bufs=4) as inp,
            tc.tile_pool(name="xe", bufs=8) as xep,
            tc.tile_pool(name="zp", bufs=8) as zp,
            tc.tile_pool(name="outp", bufs=4) as outp,
            tc.tile_pool(name="ps", bufs=2, space=bass.MemorySpace.PSUM) as psp,
        ):
            wl_sb = wpool.tile([128, NCHUNKS, O], f16)
            nc.sync.dma_start(wl_sb[:], wl_d[:])

            # Paired s-tiles: each shuffle/mul covers 1024 samples (two matmul
            # tiles) to halve DVE op count and PE supply-wait events; the four
            # PSUM accumulation chains use exactly 8 banks at bufs=2.
            W2 = 2 * S_TILE
            for t2 in range(n_stiles // 2):
                sl2 = bass.ts(t2, W2)
                xil_t = inp.tile([128, W2], f16)
                nc.sync.dma_start(xil_t[:], xil_d[:, sl2])
                ya_t = inp.tile([128, W2], f16)
                nc.sync.dma_start(ya_t[:], ya_d[:, sl2])
                yb_t = inp.tile([128, W2], f16)
                nc.sync.dma_start(yb_t[:], yb_d[:, sl2])

                psA0 = psp.tile([128, S_TILE], f32, tag="psA0")
                psB0 = psp.tile([72, S_TILE], f32, tag="psB0")
                psA1 = psp.tile([128, S_TILE], f32, tag="psA1")
                psB1 = psp.tile([72, S_TILE], f32, tag="psB1")
                ps = [psA0, psB0, psA1, psB1]
                for c in range(NCHUNKS):
                    xe = xep.tile([128, W2], f16, tag="xe")
                    nc.vector.stream_shuffle(xe[:], xil_t[:], _shuffle_mask(c))
                    z = zp.tile([128, W2], f16)
                    yt = ya_t if c < 10 else yb_t
                    eng = nc.gpsimd if c in GPSIMD_MULS else nc.vector
                    eng.tensor_mul(z[:], yt[:], xe[:])
                    first, last = c == 0, c == NCHUNKS - 1
                    for h in range(2):
                        zh = z[:, h * S_TILE : (h + 1) * S_TILE]
                        nc.tensor.matmul(
                            ps[2 * h][:], wl_sb[:, c, 0:128], zh,
                            start=first, stop=last,
                        )
                        nc.tensor.matmul(
                            ps[2 * h + 1][:], wl_sb[:, c, 128:200], zh,
                            start=first, stop=last,
                        )

                for h in range(2):
                    sl = bass.ts(2 * t2 + h, S_TILE)
                    oA = outp.tile([128, S_TILE], f16, tag="oA")
                    nc.scalar.copy(oA[:], ps[2 * h][:])
                    oB = outp.tile([72, S_TILE], f16, tag="oB")
                    nc.scalar.copy(oB[:], ps[2 * h + 1][:])
                    nc.scalar.dma_start(out_d[0:128, sl], oA[:])
                    nc.scalar.dma_start(out_d[128:200, sl], oB[:])

    nc.compile()
    return nc


def kernel(x: np.ndarray, y: np.ndarray, W: np.ndarray) -> np.ndarray:
    from concourse.bass_utils import run_bass_kernel_spmd

    assert x.shape == (BS, DIM, N) and y.shape == (BS, DIM, M)
    assert W.shape == (O, N * M)

    wl = _stage_w(W)
    x_cores = x.reshape(NCORES, S_PER_CORE, N)
    y_cores = y.reshape(NCORES, S_PER_CORE, M)

    in_maps = []
    for i in range(NCORES):
        xil, yrepa, yrepb = _stage_core_inputs(x_cores[i], y_cores[i])
        in_maps.append({"xil": xil, "yrepa": yrepa, "yrepb": yrepb, "wl": wl})

    nc = build_nc()
    res = run_bass_kernel_spmd(nc, in_maps, core_ids=list(range(NCORES)))

    outs = []
    for i in range(NCORES):
        outt = res.results[i]["outt"]  # [O, S_PER_CORE] f16
        outs.append(outt.T.astype(np.float32))  # [S_PER_CORE, O]
    return np.concatenate(outs, axis=0).reshape(BS, DIM, O)


if __name__ == "__main__":
    xs = np.random.randn(BS, DIM, N).astype(np.float32)
    ys = np.random.randn(BS, DIM, M).astype(np.float32)
    Ws = (np.random.randn(O, N * M) * (1.0 / np.sqrt(N * M))).astype(np.float32)
    out = kernel(xs, ys, Ws)
    print(out.shape, out.dtype)



# revision 2
# speedup vs baseline: 1.5273x; 1.5273x over previous
"""Trainium2 Bass kernel for nn_CINLayer: out[b,d,o] = sum_{n,m} x[b,d,n]*y[b,d,m]*W[o,n*M+m].

Strategy (8-core data parallel over batch), v2:
  Per sample s, out[s,o] = sum_k Z[k,s] * Wl[k,o] with Z[k,s] = x[s,n(k)]*y[s,m(k)].
  The 1600 (n,m) products are covered by 13 chunks of 128 rows, each chunk a
  product set P x Q so its Z is ONE elementwise multiply of two host-staged
  broadcast layouts (no on-device shuffles):
    part A (c=2a+b<10):  row r: (n,m) = (8a + r//16, 16b + r%16)
        z_c = XA_a * YA_b,  XA_a[r]=xT[8a+r//16], YA_b[r]=yT[16b+r%16]
    part B (c=10+cb):    row r: (n,m) = (16cb + r//8, 32 + r%8)  [n>=40 zeroed]
        z_c = XB_cb * YB,  XB_cb[r]=xT[16cb+r//8], YB[r]=yT[32+r%8]
  The 11 layouts are staged per 2048-sample group as one contiguous DRAM block
  ([128, 11*2048] f16, ~22KB/partition rows) so input DMA runs near peak BW.
  PE runs z-stationary matmuls: lhsT = z chunk slice [128k, 128 samples]
  (128-wide stationary -> fast weight load), moving = W chunk [128k, 200o],
  accumulating psum[128 samples, 200 o] over the 13 chunks. This avoids the
  o=200 -> 128+72 two-pass padding of the W-stationary form.
  Output: psum -> f16 SBUF tiles packing 4 sample-blocks [128, 800] -> HBM.
"""

import numpy as np

BS, DIM, N, M, O = 2048, 32, 40, 40, 200
NCORES = 8
S_PER_CORE = BS * DIM // NCORES  # 8192
NCHUNKS = 13
NLAY = 11
WG = 2048                  # samples per group
NG = S_PER_CORE // WG      # 4
BLK = 128                  # samples per matmul stationary block
NBLK_G = WG // BLK         # 16 blocks per group
F16 = np.float16

# chunk c -> (x layout index, y layout index) in the 11-layout table
# layouts: 0-4 = XA_0..4, 5-6 = YA_0..1, 7-9 = XB_0..2, 10 = YB
CHUNK_LAYS = [(c // 2, 5 + c % 2) for c in range(10)] + [(7 + cb, 10) for cb in range(3)]
# mul issue order (also PE chain uses rotations of this)
MORDER = list(range(NCHUNKS))
# chunks whose Z-multiply runs on GPSIMD instead of DVE (tune by measurement)
GPSIMD_MULS = frozenset()


def _chunk_row_to_nm(c: int, r: int):
    """Chunk c (0..12), row r (0..127) -> (n, m) or None (zero pad)."""
    if c < 10:
        return 8 * (c // 2) + r // 16, 16 * (c % 2) + r % 16
    cb = c - 10
    n = 16 * cb + r // 8
    if n >= N:
        return None
    return n, 32 + r % 8


def _stage_w(W: np.ndarray) -> np.ndarray:
    """W [O, N*M] f32 -> wl [128, NCHUNKS, O] f16 (z-stationary moving operand)."""
    Wr = W.reshape(O, N, M)
    wl = np.zeros((128, NCHUNKS, O), dtype=F16)
    for c in range(NCHUNKS):
        for r in range(128):
            nm = _chunk_row_to_nm(c, r)
            if nm is not None:
                wl[r, c, :] = Wr[:, nm[0], nm[1]].astype(F16)
    return wl


def _lay_row_maps():
    """11 layouts: list of (which, idx[128]) with idx=-1 meaning zero row."""
    r = np.arange(128)
    maps = []
    for a in range(5):
        maps.append(("x", 8 * a + r // 16))
    for b in range(2):
        maps.append(("y", 16 * b + r % 16))
    for cb in range(3):
        idx = 16 * cb + r // 8
        maps.append(("x", np.where(idx < N, idx, -1)))
    maps.append(("y", 32 + r % 8))
    return maps


_LAY_MAPS = _lay_row_maps()


def _stage_core_inputs(x_flat: np.ndarray, y_flat: np.ndarray) -> np.ndarray:
    """x_flat, y_flat [S_PER_CORE, 40] f32 -> xg [128, NG, NLAY, WG] f16."""
    xT = np.ascontiguousarray(x_flat.T).astype(F16)  # [40, S]
    yT = np.ascontiguousarray(y_flat.T).astype(F16)  # [40, S]
    src = {"x": xT, "y": yT}
    xg = np.zeros((128, NG, NLAY, WG), dtype=F16)
    for li, (which, idx) in enumerate(_LAY_MAPS):
        t = src[which]
        lay = np.where((idx >= 0)[:, None], t[np.clip(idx, 0, N - 1)], F16(0))
        xg[:, :, li, :] = lay.reshape(128, NG, WG)
    return xg


def unpack_out(outt: np.ndarray) -> np.ndarray:
    """outt [NG*4, 128, 4*O] f16 -> [S_PER_CORE, O] f32."""
    o4 = outt.reshape(NG * 4, 128, 4, O).transpose(0, 2, 1, 3)
    return o4.reshape(S_PER_CORE, O).astype(np.float32)


def build_nc(debug: bool = False):
    """Build the per-core Bass/Tile module."""
    import concourse.bass as bass
    import concourse.tile as tile
    from concourse import bacc, mybir

    f16 = mybir.dt.float16
    f32 = mybir.dt.float32

    nc = bacc.Bacc("TRN2", target_bir_lowering=False, debug=debug)

    xg_d = nc.dram_tensor("xg", [128, NG, NLAY * WG], f16, kind="ExternalInput")
    wl_d = nc.dram_tensor("wl", [128, NCHUNKS, O], f16, kind="ExternalInput")
    out_d = nc.dram_tensor("outt", [NG * 4, 128, 4 * O], f16, kind="ExternalOutput")

    with tile.TileContext(nc) as tc:
        with (
            tc.tile_pool(name="wpool", bufs=1) as wpool,
            tc.tile_pool(name="lay", bufs=2) as laypool,
            tc.tile_pool(name="zp", bufs=26) as zp,
            tc.tile_pool(name="outp", bufs=4) as outp,
            tc.tile_pool(name="ps", bufs=8, space=bass.MemorySpace.PSUM) as psp,
        ):
            wl_sb = wpool.tile([128, NCHUNKS, O], f16)
            nc.sync.dma_start(wl_sb[:], wl_d[:])

            for g in range(NG):
                lay = laypool.tile([128, NLAY * WG], f16)
                # split the group DMA so early chunks can start sooner
                nc.sync.dma_start(lay[:, 0 : 6 * WG], xg_d[:, g, 0 : 6 * WG])
                nc.sync.dma_start(lay[:, 6 * WG :], xg_d[:, g, 6 * WG :])

                z = {}
                for c in MORDER:
                    xi, yi = CHUNK_LAYS[c]
                    zc = zp.tile([128, WG], f16, tag="z")
                    eng = nc.gpsimd if c in GPSIMD_MULS else nc.vector
                    eng.tensor_mul(
                        zc[:],
                        lay[:, xi * WG : (xi + 1) * WG],
                        lay[:, yi * WG : (yi + 1) * WG],
                    )
                    z[c] = zc

                for k4 in range(4):
                    ot = outp.tile([128, 4 * O], f16)
                    for kk in range(4):
                        blk = 4 * k4 + kk
                        ps = psp.tile([128, 512], f32)
                        r0 = (g * NBLK_G + blk) % NCHUNKS
                        rot = MORDER[r0:] + MORDER[:r0]
                        sl = slice(blk * BLK, (blk + 1) * BLK)
                        for i, c in enumerate(rot):
                            nc.tensor.matmul(
                                ps[:, 0:O], z[c][:, sl], wl_sb[:, c, :],
                                start=(i == 0), stop=(i == NCHUNKS - 1),
                            )
                        nc.scalar.copy(ot[:, kk * O : (kk + 1) * O], ps[:, 0:O])
                    nc.scalar.dma_start(out_d[g * 4 + k4], ot[:])

    nc.compile()
    return nc


def prepare_in_maps(x: np.ndarray, y: np.ndarray, W: np.ndarray):
    wl = _stage_w(W)
    x_cores = x.reshape(NCORES, S_PER_CORE, N)
    y_cores = y.reshape(NCORES, S_PER_CORE, M)
    in_maps = []
    for i in range(NCORES):
        xg = _stage_core_inputs(x_cores[i], y_cores[i])
        in_maps.append({"xg": xg.reshape(128, NG, NLAY * WG), "wl": wl})
    return in_maps


def kernel(x: np.ndarray, y: np.ndarray, W: np.ndarray) -> np.ndarray:
    from concourse.bass_utils import run_bass_kernel_spmd

    assert x.shape == (BS, DIM, N) and y.shape == (BS, DIM, M)
    assert W.shape == (O, N * M)

    in_maps = prepare_in_maps(x, y, W)
    nc = build_nc()
    res = run_bass_kernel_spmd(nc, in_maps, core_ids=list(range(NCORES)))

    outs = [unpack_out(res.results[i]["outt"]) for i in range(NCORES)]
    return np.concatenate(outs, axis=0).reshape(BS, DIM, O)


if __name__ == "__main__":
    xs = np.random.randn(BS, DIM, N).astype(np.float32)
    ys = np.random.randn(BS, DIM, M).astype(np.float32)
    Ws = (np.random.randn(O, N * M) * (1.0 / np.sqrt(N * M))).astype(np.float32)
    out = kernel(xs, ys, Ws)
    print(out.shape, out.dtype)
